# revision 21
# baseline (speedup 1.0000x reference)
"""Two-layer LSTM (B=256, T=128, F=128, H=1024) + output projection on 8 TRN2 NeuronCores.

Sharding: pure data-parallel over batch (32 rows/core), weights replicated.

v2 design (vs the DMA-heavy v1):
- Per-core recurrent matmuls keep the 4-way TensorEngine column tiling
  (M=32 batch stationary, 4 gate-quadrant streams, N=512 bf16 moving).
- h-state transposes are PE identity-matmuls ([64,128] chunks -> [128,64]
  PSUM) + one DVE evac per layer -- no DMA xbar transposes in the loop.
- The i*g / f*c partition alignment is done by PE shift-accumulate
  matmuls into a PSUM-resident cell state (c never leaves PSUM), killing
  the per-step SBUF->SBUF shift DMA.
- Layer-1 gate biases are pre-accumulated into PSUM by K=1 ones-matmuls
  so ACT reads gate pre-activations straight from PSUM.
- The time loop is For_i_unrolled(max_unroll=8) to amortize the ~2us
  Tile back-edge barrier.
- Layer-0 input projection (x@Wih0^T + b0) precomputed in phase 1 at
  full PE width, streamed per step ([128,1024] bf16 tile, 2 bufs).
- Output projection: phase 3 accumulates into one pinned PSUM bank over
  256 contraction chunks (DMA-transposed h1 history tiles).
"""

import numpy as np
import ml_dtypes

B, T, F, H, O = 256, 128, 128, 1024, 128
NCORES = 8
BL = B // NCORES  # 32
KH = H // 128     # 8 contraction chunks
G4 = 4 * H        # 4096

_cache = {}
_NSTEPS = T          # debug: run fewer recurrence steps
_LOOP_MODE = "unroll"  # "unroll" (For_i_unrolled), "static" (python loop)
_PHASES = (1, 2, 3)  # debug: which phases to emit
_PARTS = frozenset(("xp", "gates0", "bias1", "gates1hh", "cell0", "tail0",
                    "trans0", "gates1ih", "cell1", "tail1", "trans1", "hist"))


def _build():
    import concourse.bass as bass
    import concourse.tile as tile
    import concourse.mybir as mybir
    from concourse import bacc
    from concourse.bass import ds

    F32, BF16 = mybir.dt.float32, mybir.dt.bfloat16
    AF = mybir.ActivationFunctionType
    MULT, ADD = mybir.AluOpType.mult, mybir.AluOpType.add

    nc = bacc.Bacc("TRN2", target_bir_lowering=False, debug=False,
                   num_devices=NCORES, dynamic_dma_scratch_size=4096)

    def din(name, shape, dt):
        return nc.dram_tensor(name, shape, dt, kind="ExternalInput").ap()

    xT_d = din("xT", [128, T * BL], BF16)
    whh0T_d = din("whh0T", [128, KH * G4], BF16)
    wih1T_d = din("wih1T", [128, KH * G4], BF16)
    whh1T_d = din("whh1T", [128, KH * G4], BF16)
    wih0T_d = din("wih0T", [128, G4], BF16)
    b0r_d = din("b0r", [128, G4], BF16)       # b0 replicated across rows
    b1q_d = din("b1q", [128, 1024], BF16)     # b1 in quadrant layout
    ident_d = din("ident", [128, 64], BF16)    # I64 in both partition halves
    identF_d = din("identF", [128, 128], BF16)  # full I128
    woS_d = din("woS", [256, 128, O], BF16)
    h0T_d = din("h0T", [128, KH * BL], BF16)
    h1T_d = din("h1T", [128, KH * BL], BF16)
    c0_d = din("c0", [64, 512], F32)
    c1_d = din("c1", [64, 512], F32)
    out_d = nc.dram_tensor("out", [T, O], F32, kind="ExternalOutput").ap()

    colA = lambda q: 512 * q           # i[h0] i[h1] f[h0] f[h1]
    colB = lambda q: 2048 + 512 * q    # g[h0] g[h1] o[h0] o[h1]

    with tile.TileContext(nc) as tc:
        with tc.tile_pool(name="dram", bufs=1, space="DRAM") as dp:
            # xp[t, part=(q,b), bank*512+j] : per-step gate quadrant layout
            xp_d = dp.tile([T, 128, 1024], BF16)
            # h1 history: hist[t, 32a+b, j] = bf16(h1(t))[b, 512a+j]
            hist_d = dp.tile([T, 64, 512], BF16)

            with tc.tile_pool(name="wpA", bufs=1) as wpA:
                # loaded first so the DMAs overlap phase-1 compute
                wih1T = wpA.tile([128, KH * G4], BF16)
                whh1T = wpA.tile([128, KH * G4], BF16)
                b1q = wpA.tile([128, 1024], BF16)
                ident = wpA.tile([128, 64], BF16)
                h0T = wpA.tile([128, KH * BL], BF16)
                h1T = wpA.tile([128, KH * BL], BF16)
                weng = [nc.sync, nc.scalar, nc.gpsimd]
                for k in range(KH):
                    weng[k % 3].dma_start(wih1T[:, ds(k * G4, G4)],
                                          wih1T_d[:, ds(k * G4, G4)])
                    weng[(k + 1) % 3].dma_start(whh1T[:, ds(k * G4, G4)],
                                                whh1T_d[:, ds(k * G4, G4)])
                nc.sync.dma_start(b1q[:], b1q_d[:])
                nc.sync.dma_start(ident[:], ident_d[:])
                nc.sync.dma_start(h0T[:], h0T_d[:])
                nc.sync.dma_start(h1T[:], h1T_d[:])

                # ---- phase 1: precompute layer-0 input projection ----
                if 1 in _PHASES:
                    with tc.tile_pool(name="prepool", bufs=1) as pp, \
                         tc.tile_pool(name="prepsum", bufs=2, space="PSUM") as pps:
                        xTf = pp.tile([128, T * BL], BF16)
                        wih0T = pp.tile([128, G4], BF16)
                        b0r = pp.tile([128, G4], BF16)
                        nc.sync.dma_start(xTf[:], xT_d[:])
                        nc.scalar.dma_start(wih0T[:], wih0T_d[:])
                        nc.scalar.dma_start(b0r[:], b0r_d[:])
                        for m in range(T * BL // 128):   # 32 token chunks
                            for g in range(8):           # gate-column chunks
                                pspre = pps.tile([128, 512], F32, name="pspre",
                                                 tag="pspre")
                                nc.tensor.matmul(pspre[:],
                                                 xTf[:, ds(128 * m, 128)],
                                                 wih0T[:, ds(512 * g, 512)],
                                                 start=True, stop=True,
                                                 skip_group_check=True)
                                evac = pp.tile([128, 512], BF16, name="evac",
                                               tag="evac", bufs=3)
                                nc.vector.tensor_tensor(
                                    evac[:], pspre[:],
                                    b0r[:, ds(512 * g, 512)], ADD)
                                bank, q = g // 4, g % 4
                                eng = nc.sync if (m + g) % 2 == 0 else nc.scalar
                                eng.dma_start(
                                    xp_d[ds(4 * m, 4), ds(32 * q, 32),
                                         ds(512 * bank, 512)],
                                    evac[:])

                # ---- phase 2: recurrence ----
                with tc.tile_pool(name="wpB", bufs=1) as wpB, \
                     tc.tile_pool(name="cp", bufs=1) as cp, \
                     tc.tile_pool(name="xpp", bufs=1) as xpp, \
                     tc.tile_pool(name="gp", bufs=1, space="PSUM") as gp:

                    whh0T = wpB.tile([128, KH * G4], BF16)
                    for k in range(KH):
                        weng[k % 3].dma_start(whh0T[:, ds(k * G4, G4)],
                                              whh0T_d[:, ds(k * G4, G4)])

                    # cell temps (per layer; bf16 intermediates)
                    sA0 = cp.tile([128, 512], BF16)
                    ga0 = cp.tile([128, 512], BF16)
                    P0 = cp.tile([128, 512], BF16)
                    sA1 = cp.tile([128, 512], BF16)
                    ga1 = cp.tile([128, 512], BF16)
                    P1 = cp.tile([128, 512], BF16)
                    stgA = cp.tile([128, 512], BF16)
                    stgB = cp.tile([128, 512], BF16)

                    psA0 = gp.tile([128, 512], F32)
                    psB0 = gp.tile([128, 512], F32)
                    psC0 = gp.tile([128, 512], F32)
                    psT0 = gp.tile([128, 512], F32)
                    psA1 = gp.tile([128, 512], F32)
                    psB1 = gp.tile([128, 512], F32)
                    psC1 = gp.tile([128, 512], F32)
                    psT1 = gp.tile([128, 512], F32)

                    # init c states into PSUM (DVE write)
                    with tc.tile_pool(name="initp", bufs=1) as ip:
                        c0sb = ip.tile([128, 512], F32)
                        c1sb = ip.tile([128, 512], F32)
                        nc.sync.dma_start(c0sb[ds(64, 64), :], c0_d[:])
                        nc.sync.dma_start(c1sb[ds(64, 64), :], c1_d[:])
                        nc.vector.tensor_copy(psC0[ds(64, 64), :],
                                              c0sb[ds(64, 64), :])
                        nc.vector.tensor_copy(psC1[ds(64, 64), :],
                                              c1sb[ds(64, 64), :])

                    def gate_rounds(ps, colf, hT_sb, wT_sb, start, stop):
                        for k in range(KH):
                            for q in range(4):
                                nc.tensor.matmul(
                                    ps[ds(32 * q, 32), :],
                                    hT_sb[:, ds(32 * k, 32)],
                                    wT_sb[:, ds(k * G4 + colf(q), 512)],
                                    start=(start and k == 0),
                                    stop=(stop and k == KH - 1),
                                    tile_position=(0, 32 * q),
                                    skip_group_check=True)

                    def cell_tail(P, ga, sA, psC):
                        # P[0:64]=i*g and P[64:128]=f*c already written.
                        # shift i*g down into the PSUM c bank, then DVE
                        # in-place add of f*c (single-tile-position groups
                        # only -- multi-row-group accumulation faults here).
                        nc.tensor.matmul(psC[ds(64, 64), :],
                                         ident[ds(0, 64), :],
                                         P[ds(0, 64), :],
                                         start=True, stop=True,
                                         tile_position=(0, 64),
                                         skip_group_check=True)
                        nc.vector.tensor_tensor(psC[ds(64, 64), :],
                                                psC[ds(64, 64), :],
                                                P[ds(64, 64), :], ADD)
                        nc.scalar.activation(sA[ds(64, 64), :],
                                             psC[ds(64, 64), :], AF.Tanh)
                        nc.vector.tensor_tensor(P[ds(64, 64), :],
                                                ga[ds(64, 64), :],
                                                sA[ds(64, 64), :], MULT)

                    def transposes(psT, P, hT):
                        for m in range(4):
                            nc.tensor.matmul(psT[:, ds(64 * m, 64)],
                                             P[ds(64, 64), ds(128 * m, 128)],
                                             ident[ds(64, 64), :],
                                             start=True, stop=True,
                                             tile_position=(64, 0),
                                             skip_group_check=True)
                        nc.vector.tensor_copy(hT[:], psT[:, ds(0, 256)])

                    PT = _PARTS

                    def emit_step(tv):
                        xp = xpp.tile([128, 1024], BF16, name="xp", tag="xp",
                                      bufs=2)
                        if "xp" in PT:
                            nc.scalar.dma_start(xp[:], xp_d[tv])
                        # layer-0 gates (bank B first: its ACT work is deeper)
                        if "gates0" in PT:
                            gate_rounds(psB0, colB, h0T, whh0T, True, True)
                            gate_rounds(psA0, colA, h0T, whh0T, True, True)
                        # layer-1 hidden part (PE filler during cell0)
                        if "gates1hh" in PT:
                            gate_rounds(psB1, colB, h1T, whh1T, True, False)
                            gate_rounds(psA1, colA, h1T, whh1T, True, False)
                        # ---- cell 0 ----
                        if "cell0" in PT:
                            nc.vector.tensor_tensor(stgB[:], psB0[:],
                                                    xp[:, ds(512, 512)], ADD)
                            nc.scalar.activation(ga0[ds(0, 64), :],
                                                 stgB[ds(0, 64), :], AF.Tanh)
                            nc.scalar.activation(ga0[ds(64, 64), :],
                                                 stgB[ds(64, 64), :],
                                                 AF.Sigmoid)
                            nc.vector.tensor_tensor(stgA[:], psA0[:],
                                                    xp[:, ds(0, 512)], ADD)
                            nc.scalar.activation(sA0[:], stgA[:], AF.Sigmoid)
                            nc.vector.tensor_tensor(P0[ds(0, 64), :],
                                                    sA0[ds(0, 64), :],
                                                    ga0[ds(0, 64), :], MULT)
                            nc.vector.tensor_tensor(P0[ds(64, 64), :],
                                                    sA0[ds(64, 64), :],
                                                    psC0[ds(64, 64), :], MULT)
                        if "tail0" in PT:
                            cell_tail(P0, ga0, sA0, psC0)
                        if "trans0" in PT:
                            transposes(psT0, P0, h0T)
                        # layer-1 input part (needs new h0T)
                        if "gates1ih" in PT:
                            gate_rounds(psB1, colB, h0T, wih1T, False, True)
                            gate_rounds(psA1, colA, h0T, wih1T, False, True)
                        # ---- cell 1 ----
                        if "cell1" in PT:
                            nc.vector.tensor_tensor(stgB[:], psB1[:],
                                                    b1q[:, ds(512, 512)], ADD)
                            nc.scalar.activation(ga1[ds(0, 64), :],
                                                 stgB[ds(0, 64), :], AF.Tanh)
                            nc.scalar.activation(ga1[ds(64, 64), :],
                                                 stgB[ds(64, 64), :],
                                                 AF.Sigmoid)
                            nc.vector.tensor_tensor(stgA[:], psA1[:],
                                                    b1q[:, ds(0, 512)], ADD)
                            nc.scalar.activation(sA1[:], stgA[:], AF.Sigmoid)
                            nc.vector.tensor_tensor(P1[ds(0, 64), :],
                                                    sA1[ds(0, 64), :],
                                                    ga1[ds(0, 64), :], MULT)
                            nc.vector.tensor_tensor(P1[ds(64, 64), :],
                                                    sA1[ds(64, 64), :],
                                                    psC1[ds(64, 64), :], MULT)
                        if "tail1" in PT:
                            cell_tail(P1, ga1, sA1, psC1)
                        if "trans1" in PT:
                            transposes(psT1, P1, h1T)
                        if "hist" in PT:
                            nc.sync.dma_start(hist_d[tv], P1[ds(64, 64), :])

                    if _LOOP_MODE == "static":
                        for tvv in range(_NSTEPS):
                            emit_step(tvv)
                    else:
                        tc.For_i_unrolled(0, _NSTEPS, 1, emit_step,
                                          max_unroll=8)

            # ---- phase 3: output projection  partial[t,o] ----
            # hist tiles are [t, h]; lhsT needs [h, t] -- transpose on the
            # PE via identity matmuls (DMA xbar transpose is avoided: it
            # hangs on this runtime).
            with tc.tile_pool(name="fpool", bufs=8) as fp, \
                 tc.tile_pool(name="fpsum", bufs=1, space="PSUM") as fps, \
                 tc.tile_pool(name="fpsT", bufs=2, space="PSUM") as fpsT:
                outp = fps.tile([128, O], F32)
                identF = fp.tile([128, 128], BF16)
                nc.sync.dma_start(identF[:], identF_d[:])
                nkp = 256 if 3 in _PHASES else 0
                for kp in range(nkp):
                    b, sub = kp // 8, kp % 8
                    a, m = sub // 4, sub % 4
                    of = fp.tile([128, 128], BF16, name="of", tag="of")
                    nc.sync.dma_start(of[:],
                                      hist_d[:, 32 * a + b, ds(128 * m, 128)])
                    psX = fpsT.tile([128, 128], F32, name="psX", tag="psX")
                    nc.tensor.matmul(psX[:], of[:], identF[:],
                                     start=True, stop=True,
                                     skip_group_check=True)
                    ofT = fp.tile([128, 128], BF16, name="ofT", tag="ofT")
                    nc.vector.tensor_copy(ofT[:], psX[:])
                    wos = fp.tile([128, O], BF16, name="wos", tag="wos")
                    nc.scalar.dma_start(wos[:], woS_d[kp])
                    nc.tensor.matmul(outp[:], ofT[:], wos[:],
                                     start=(kp == 0), stop=(kp == nkp - 1),
                                     skip_group_check=True)
                oev = fp.tile([128, O], F32)
                if nkp:
                    nc.vector.tensor_copy(oev[:], outp[:])
                else:
                    nc.vector.memset(oev[:], 0.0)
                nc.sync.dma_start(out_d[:], oev[:])

    nc.compile()
    return nc


def _prep(inputs):
    bf = ml_dtypes.bfloat16

    def wT2(w):  # [4H, 1024] -> [128, KH*G4], chunk order k' = 2m + a
        return np.ascontiguousarray(
            w.T.reshape(2, 4, 128, G4).transpose(2, 1, 0, 3).reshape(128, KH * G4)
        ).astype(bf)

    def hT2(h):  # [32, 1024] -> [128, KH*32], chunk order k' = 2m + a
        return np.ascontiguousarray(
            h.T.reshape(2, 4, 128, BL).transpose(2, 1, 0, 3).reshape(128, KH * BL)
        ).astype(bf)

    b1 = (np.asarray(inputs["bih1"], np.float32)
          + np.asarray(inputs["bhh1"], np.float32))
    # quadrant layout: rows 32q+b get bias chunk q (bank A cols 0:512 = i/f,
    # bank B cols 512:1024 = g/o)
    bA = b1[0:2048].reshape(4, 512)
    bB = b1[2048:4096].reshape(4, 512)
    b1q = np.zeros((128, 1024), np.float32)
    for q in range(4):
        b1q[32 * q:32 * q + 32, 0:512] = bA[q]
        b1q[32 * q:32 * q + 32, 512:1024] = bB[q]
    ident = np.zeros((128, 64), np.float32)
    for p in range(128):
        ident[p, p % 64] = 1.0

    shared = {
        "whh0T": wT2(np.asarray(inputs["Whh0"], np.float32)),
        "wih1T": wT2(np.asarray(inputs["Wih1"], np.float32)),
        "whh1T": wT2(np.asarray(inputs["Whh1"], np.float32)),
        "wih0T": np.ascontiguousarray(
            np.asarray(inputs["Wih0"], np.float32).T).astype(bf),
        "b0r": np.ascontiguousarray(np.broadcast_to(
            (np.asarray(inputs["bih0"], np.float32)
             + np.asarray(inputs["bhh0"], np.float32))[None, :],
            (128, G4))).astype(bf),
        "b1q": b1q.astype(bf),
        "ident": ident.astype(bf),
        "identF": np.eye(128, dtype=np.float32).astype(bf),
    }
    WoT = np.asarray(inputs["Wout"], np.float32).T  # [T*H, O]
    xr = np.asarray(inputs["batch"], np.float32).reshape(T, B, F)
    in_maps = []
    for c in range(NCORES):
        sl = slice(BL * c, BL * (c + 1))
        m = dict(shared)
        m["woS"] = np.ascontiguousarray(
            WoT[32768 * (c % 4):32768 * (c % 4) + 32768].reshape(256, 128, O)
        ).astype(bf)
        m["xT"] = np.ascontiguousarray(
            xr[:, sl, :].transpose(2, 0, 1).reshape(F, T * BL)).astype(bf)
        m["h0T"] = hT2(np.asarray(inputs["h00"], np.float32)[sl])
        m["h1T"] = hT2(np.asarray(inputs["h01"], np.float32)[sl])
        for nm, csrc in (("c0", "c00"), ("c1", "c01")):
            cc = np.asarray(inputs[csrc], np.float32)[sl]  # [32, 1024]
            m[nm] = np.ascontiguousarray(
                cc.reshape(BL, 2, 512).transpose(1, 0, 2).reshape(64, 512))
        in_maps.append(m)
    return in_maps


def kernel(**inputs):
    from concourse import bass_utils

    if "nc" not in _cache:
        _cache["nc"] = _build()
    nc = _cache["nc"]
    in_maps = _prep(inputs)
    r = None
    for attempt in range(3):
        try:
            r = bass_utils.run_bass_kernel_spmd(nc, in_maps,
                                                core_ids=list(range(NCORES)))
            break
        except Exception:
            if attempt == 2:
                raise
    parts = np.stack([r.results[c]["out"] for c in range(NCORES)])  # [8, T, O]
    bout = np.asarray(inputs["bout"], np.float32)
    out = np.empty((B, O), np.float32)
    out[0::2] = parts[0:4].sum(axis=0) + bout   # rows 2t
    out[1::2] = parts[4:8].sum(axis=0) + bout   # rows 2t+1
    return out
